# revision 15
# baseline (speedup 1.0000x reference)
"""Trainium2 Bass kernel for nn_AttnFreeLayer (linear-attention-style layer).

Computes, for inputs q,k,v [B,S,D] and weights Wq,Wk,Wv [E,D] (E=D):
    q_in = elu(q @ Wq^T) + 1
    k_in = elu(k @ Wk^T) + 1
    v_in = v @ Wv^T
    kv_in = k_in * v_in
    out = q_in * (kv_in + cumsum_s(kv_in)) / cumsum_s(k_in)

Sharding: 8 cores = 4 batches x 2 halves of the output dim E; no
collectives. Each core computes out[b, e0:e0+512, :] in a TRANSPOSED
[e, s] layout: the projection matmuls put W chunks stationary and x^T
moving, so outputs land with e on partitions and s on the free dim.
The seq-cumsum then runs along the free dimension via the DVE
tensor_tensor_scan instruction (chained across s-chunks through its
`initial` operand) - no triangular-matmul cumsum, no carry matmuls,
no PSUM-tail copies.

Precision: hybrid fp16/fp8. The output is only sensitive to projection
error at tiny s (out ~ 2*q_in*v_in at s=0, decaying like 1/k_prefix),
so the first 512 seq positions use fp16 matmuls and the remaining 7680
use fp8-e4m3 DoubleRow matmuls (2x PE throughput, K=256 per
instruction). Host-simulated max rel err of this scheme is ~1.0e-3 vs
the 2e-2 gate. fp8 weights are pre-scaled by 16 (avoids e4m3
subnormals); the downstream ACT ops undo it via their free `scale`.

elu(x)+1 = min(exp(x), 1) + relu(x); exp/relu/copy run on ACT (one
table set), the min+add folds into a single DVE scalar_tensor_tensor,
kv-mul and num-add run on GPSIMD, scans/reciprocal/final muls on DVE.
Per [128e, 512s] unit: PE ~2.6us(fp16)/~1.3-2.6us(fp8), ACT ~2.6us,
DVE ~2.7us, GPSIMD ~2.2us -> all engines roughly balanced.
"""

import sys

for _p in ("/opt/trn_rl_repo",):
    if _p not in sys.path:
        sys.path.insert(0, _p)

from contextlib import ExitStack

import numpy as np
import ml_dtypes

import concourse.bass as bass
import concourse.tile as tile
from concourse import bacc
from concourse import mybir
from concourse.alu_op_type import AluOpType
from concourse.bass_utils import run_bass_kernel_spmd

FP8 = mybir.dt.float8e4
FP16 = mybir.dt.float16
FP32 = mybir.dt.float32
AF = mybir.ActivationFunctionType
DR = mybir.MatmulPerfMode.DoubleRow

B, S, D, E = 4, 8192, 1024, 1024
NCORES = 8
EH = E // 2  # e-half per core
P = 128  # partition block
SC = 512  # s-chunk width (PSUM bank = 512 fp32)
N_SC = S // SC  # 16
N_EC = EH // P  # 4
ND = D // P  # 8 contraction chunks
WS = 16.0  # fp8 weight prescale (undone by ACT `scale`)


def build_nc(repeat=1, debug=False, scan_mode="scan", gps=False, div=False):
    """scan_mode: "scan" (tensor_tensor_scan) or "copy" (timing-only probe:
    replaces the cumsums with tensor_copy - numerically WRONG, bench only).
    gps: offload kv-mul/num-add to GPSIMD (measured: its SBUF-port
    contention with DVE costs more than the offload saves - keep False).
    div: single tensor_tensor divide instead of reciprocal+mul (REJECTED by
    walrus codegen: divide is not a valid TensorTensorArith ALU op on TRN2 -
    keep False)."""
    nc = bacc.Bacc("TRN2", target_bir_lowering=False, debug=debug)

    x8 = [
        nc.declare_dram_parameter(f"x{n}8", [D, S], FP8, isOutput=False)
        for n in ("q", "k", "v")
    ]
    x16 = [
        nc.declare_dram_parameter(f"x{n}16", [D, SC], FP16, isOutput=False)
        for n in ("q", "k", "v")
    ]
    w16 = [
        nc.declare_dram_parameter(f"w{n}16", [D, EH], FP16, isOutput=False)
        for n in ("q", "k", "v")
    ]
    w8 = [
        nc.declare_dram_parameter(f"w{n}8", [D, EH], FP8, isOutput=False)
        for n in ("q", "k", "v")
    ]
    outp = nc.declare_dram_parameter("out", [EH, S], FP16, isOutput=True)

    with tile.TileContext(nc) as tc, ExitStack() as ctx:
        wpool = ctx.enter_context(tc.tile_pool(name="w", bufs=1))
        x8pool = ctx.enter_context(tc.tile_pool(name="x8", bufs=2))
        apool = ctx.enter_context(tc.tile_pool(name="act", bufs=3))
        vpool = ctx.enter_context(tc.tile_pool(name="vv", bufs=3))
        cpool = ctx.enter_context(tc.tile_pool(name="cum", bufs=6))
        opool = ctx.enter_context(tc.tile_pool(name="out", bufs=3))
        pp = ctx.enter_context(tc.tile_pool(name="pqk", bufs=3, space="PSUM"))
        pvp = ctx.enter_context(tc.tile_pool(name="pv", bufs=2, space="PSUM"))

        # --- resident weights + first-chunk fp16 x ---
        w16_t, w8_t, x16_t = [], [], []
        for i in range(3):
            t = wpool.tile([P, ND, EH], FP16, tag=f"w16_{i}")
            nc.sync.dma_start(
                out=t[:], in_=w16[i][:].rearrange("(j p) e -> p j e", p=P)
            )
            w16_t.append(t)
            t8 = wpool.tile([P, ND, EH], FP8, tag=f"w8_{i}")
            nc.sync.dma_start(
                out=t8[:], in_=w8[i][:].rearrange("(j p) e -> p j e", p=P)
            )
            w8_t.append(t8)
            tx = wpool.tile([P, ND, SC], FP16, tag=f"x16_{i}")
            nc.sync.dma_start(
                out=tx[:], in_=x16[i][:].rearrange("(j p) s -> p j s", p=P)
            )
            x16_t.append(tx)

        carry_k = [None] * N_EC
        carry_kv = [None] * N_EC

        def emit_unit(sc, ec, xts, fp8):
            e0 = ec * P
            sscale = (1.0 / WS) if fp8 else 1.0
            pqk = pp.tile([P, 2 * SC], FP32, tag="pqk")
            pv_ = pvp.tile([P, SC], FP32, tag="pv")
            for i in range(3):
                dst = pqk[:, i * SC : (i + 1) * SC] if i < 2 else pv_[:]
                if fp8:
                    for j in range(ND // 2):
                        nc.tensor.matmul(
                            dst,
                            lhsT=w8_t[i][:, 2 * j : 2 * j + 2, e0 : e0 + P],
                            rhs=xts[i][:, 2 * j : 2 * j + 2, :],
                            start=(j == 0),
                            stop=(j == ND // 2 - 1),
                            perf_mode=DR,
                            skip_group_check=True,
                        )
                else:
                    for j in range(ND):
                        nc.tensor.matmul(
                            dst,
                            lhsT=w16_t[i][:, j, e0 : e0 + P],
                            rhs=xts[i][:, j, :],
                            start=(j == 0),
                            stop=(j == ND - 1),
                            skip_group_check=True,
                        )
            # elu(x)+1 = min(exp(x),1) + relu(x) over merged q|k
            ex = apool.tile([P, 2 * SC], FP16, tag="ex")
            nc.scalar.activation(ex[:], pqk[:], AF.Exp, scale=sscale)
            rp = apool.tile([P, 2 * SC], FP16, tag="rp")
            nc.scalar.activation(rp[:], pqk[:], AF.Relu, scale=sscale)
            v1 = vpool.tile([P, SC], FP16, tag="v1")
            nc.scalar.activation(v1[:], pv_[:], AF.Copy, scale=sscale)
            qk1 = apool.tile([P, 2 * SC], FP16, tag="qk1")
            nc.vector.scalar_tensor_tensor(
                qk1[:], ex[:], 1.0, rp[:], op0=AluOpType.min, op1=AluOpType.add
            )
            q1 = qk1[:, 0:SC]
            k1 = qk1[:, SC : 2 * SC]
            kv = vpool.tile([P, SC], FP16, tag="kv")
            (nc.gpsimd if gps else nc.vector).tensor_mul(kv[:], k1, v1[:])
            # inclusive cumsums along s (free dim), chained across s-chunks
            ck = cpool.tile([P, SC], FP16, tag="ck")
            ckv = cpool.tile([P, SC], FP16, tag="ckv")
            if scan_mode == "scan":
                ik = 0.0 if sc == 0 else carry_k[ec][:, SC - 1 : SC]
                ikv = 0.0 if sc == 0 else carry_kv[ec][:, SC - 1 : SC]
                nc.vector.tensor_tensor_scan(
                    ck[:], k1, k1, ik, op0=AluOpType.add, op1=AluOpType.bypass
                )
                nc.vector.tensor_tensor_scan(
                    ckv[:], kv[:], kv[:], ikv, op0=AluOpType.add, op1=AluOpType.bypass
                )
            else:  # timing probe only
                nc.vector.tensor_copy(ck[:], k1)
                nc.vector.tensor_copy(ckv[:], kv[:])
            carry_k[ec], carry_kv[ec] = ck, ckv
            # out = q1 * (kv + ckv) / ck
            num = vpool.tile([P, SC], FP16, tag="num")
            (nc.gpsimd if gps else nc.vector).tensor_add(num[:], ckv[:], kv[:])
            t1 = vpool.tile([P, SC], FP16, tag="t1")
            nc.vector.tensor_mul(t1[:], q1, num[:])
            ot = opool.tile([P, SC], FP16, tag="ot")
            if div:
                nc.vector.tensor_tensor(ot[:], t1[:], ck[:], AluOpType.divide)
            else:
                den = vpool.tile([P, SC], FP16, tag="den")
                with nc.allow_low_precision(reason="1/k_prefix; 5e-4 rel ok"):
                    nc.vector.reciprocal(den[:], ck[:])
                nc.vector.tensor_mul(ot[:], t1[:], den[:])
            nc.sync.dma_start(
                out=outp[e0 : e0 + P, sc * SC : (sc + 1) * SC], in_=ot[:]
            )

        def main_body():
            for sc in range(N_SC):
                if sc == 0:
                    xts, fp8 = x16_t, False
                else:
                    xts = []
                    for i in range(3):
                        t = x8pool.tile([P, ND, SC], FP8, tag=f"x8{i}")
                        nc.sync.dma_start(
                            out=t[:],
                            in_=x8[i][:, sc * SC : (sc + 1) * SC].rearrange(
                                "(j p) s -> p j s", p=P
                            ),
                        )
                        xts.append(t)
                    fp8 = True
                for ec in range(N_EC):
                    emit_unit(sc, ec, xts, fp8)

        if repeat == 1:
            main_body()
        else:
            with tc.For_i(0, repeat, 1):
                main_body()

    nc.compile()
    return nc


def _e4m3(x):
    return np.clip(x, -240, 240).astype(ml_dtypes.float8_e4m3)


def _host_prep(v, k, q, Wq, Wk, Wv):
    """Build the 8 per-core input maps (x^T computed once per batch)."""
    xT8, xT16 = {}, {}
    for b in range(B):
        for n, x in (("q", q), ("k", k), ("v", v)):
            t = np.ascontiguousarray(x[b].T)  # [D, S] fp32
            xT8[(b, n)] = _e4m3(t)
            xT16[(b, n)] = t[:, :SC].astype(np.float16)
    in_maps = []
    for c in range(NCORES):
        b, h = c // 2, c % 2
        e0 = h * EH
        m = {}
        for n in ("q", "k", "v"):
            m[f"x{n}8"] = xT8[(b, n)]
            m[f"x{n}16"] = xT16[(b, n)]
        for n, W in (("q", Wq), ("k", Wk), ("v", Wv)):
            wt = np.ascontiguousarray(W.T[:, e0 : e0 + EH])
            m[f"w{n}16"] = wt.astype(np.float16)
            m[f"w{n}8"] = _e4m3(wt * WS)
        in_maps.append(m)
    return in_maps


_NC_CACHE = None


def _get_nc():
    global _NC_CACHE
    if _NC_CACHE is None:
        _NC_CACHE = build_nc()
    return _NC_CACHE


def run_spmd(v, k, q, Wq, Wk, Wv, **kwargs):
    """Run on 8 cores; returns (assembled output [B,S,E] fp32, raw results)."""
    nc = _get_nc()
    in_maps = _host_prep(v, k, q, Wq, Wk, Wv)
    res = run_bass_kernel_spmd(nc, in_maps, core_ids=list(range(NCORES)), **kwargs)
    full = np.empty((B, S, E), dtype=np.float32)
    for c in range(NCORES):
        b, h = c // 2, c % 2
        full[b, :, h * EH : (h + 1) * EH] = res.results[c]["out"].T.astype(np.float32)
    return full, res


def kernel(v, k, q, Wq, Wk, Wv):
    v, k, q, Wq, Wk, Wv = (
        np.asarray(a, dtype=np.float32) for a in (v, k, q, Wq, Wk, Wv)
    )
    full, _ = run_spmd(v, k, q, Wq, Wk, Wv)
    return full


# revision 17
# speedup vs baseline: 1.3841x; 1.3841x over previous
"""Trainium2 Bass kernel for nn_AttnFreeLayer (linear-attention-style layer).

Computes, for inputs q,k,v [B,S,D] and weights Wq,Wk,Wv [E,D] (E=D):
    q_in = elu(q @ Wq^T) + 1
    k_in = elu(k @ Wk^T) + 1
    v_in = v @ Wv^T
    kv_in = k_in * v_in
    out = q_in * (kv_in + cumsum_s(kv_in)) / cumsum_s(k_in)

Sharding: 8 cores = 4 batches x 2 halves of the output dim E; no
collectives. Each core computes out[b, e0:e0+512, :] in a TRANSPOSED
[e, s] layout: the projection matmuls put W chunks stationary and x^T
moving, so outputs land with e on partitions and s on the free dim.
The seq-cumsum then runs along the free dimension via the DVE
tensor_tensor_scan instruction (chained across s-chunks through its
`initial` operand) - no triangular-matmul cumsum, no carry matmuls,
no PSUM-tail copies.

Precision: hybrid fp16/fp8. The output is only sensitive to projection
error at tiny s (out ~ 2*q_in*v_in at s=0, decaying like 1/k_prefix),
so the first 512 seq positions use fp16 matmuls and the remaining 7680
use fp8-e4m3 DoubleRow matmuls (2x PE throughput, K=256 per
instruction). Host-simulated max rel err of this scheme is ~1.0e-3 vs
the 2e-2 gate. fp8 weights are pre-scaled by 16 (avoids e4m3
subnormals); the downstream ACT ops undo it via their free `scale`.

elu(x)+1 = min(exp(x), 1) + relu(x); exp/relu/copy run on ACT (one
table set, loaded once), the min+add folds into a single DVE
scalar_tensor_tensor; kv-mul, scans, num-add, reciprocal and the final
muls all run on DVE. Measured notes: GPSIMD offload of the elementwise
ops HURTS (SBUF-port contention with DVE, +~100us) and tensor_tensor
divide is not a valid TRN2 ISA op, hence gps=False / div=False.
Measured ~480-630us/pass (bench noise +-80us) at rel err 1.29e-3.
"""

import sys

for _p in ("/opt/trn_rl_repo",):
    if _p not in sys.path:
        sys.path.insert(0, _p)

from contextlib import ExitStack

import numpy as np
import ml_dtypes

import concourse.bass as bass
import concourse.tile as tile
from concourse import bacc
from concourse import mybir
from concourse.alu_op_type import AluOpType
from concourse.bass_utils import run_bass_kernel_spmd

FP8 = mybir.dt.float8e4
FP16 = mybir.dt.float16
FP32 = mybir.dt.float32
AF = mybir.ActivationFunctionType
DR = mybir.MatmulPerfMode.DoubleRow

B, S, D, E = 4, 8192, 1024, 1024
NCORES = 8
EH = E // 2  # e-half per core
P = 128  # partition block
SC = 512  # s-chunk width (PSUM bank = 512 fp32)
N_SC = S // SC  # 16
N_EC = EH // P  # 4
ND = D // P  # 8 contraction chunks
WS = 16.0  # fp8 weight prescale (undone by ACT `scale`)


def build_nc(repeat=1, debug=False, scan_mode="scan", gps=False, div=False):
    """scan_mode: "scan" (tensor_tensor_scan) or "copy" (timing-only probe:
    replaces the cumsums with tensor_copy - numerically WRONG, bench only).
    gps: offload kv-mul/num-add to GPSIMD (measured: its SBUF-port
    contention with DVE costs more than the offload saves - keep False).
    div: single tensor_tensor divide instead of reciprocal+mul (REJECTED by
    walrus codegen: divide is not a valid TensorTensorArith ALU op on TRN2 -
    keep False)."""
    nc = bacc.Bacc("TRN2", target_bir_lowering=False, debug=debug)

    x8 = [
        nc.declare_dram_parameter(f"x{n}8", [D, S], FP8, isOutput=False)
        for n in ("q", "k", "v")
    ]
    x16 = [
        nc.declare_dram_parameter(f"x{n}16", [D, SC], FP16, isOutput=False)
        for n in ("q", "k", "v")
    ]
    w16 = [
        nc.declare_dram_parameter(f"w{n}16", [D, EH], FP16, isOutput=False)
        for n in ("q", "k", "v")
    ]
    w8 = [
        nc.declare_dram_parameter(f"w{n}8", [D, EH], FP8, isOutput=False)
        for n in ("q", "k", "v")
    ]
    outp = nc.declare_dram_parameter("out", [EH, S], FP16, isOutput=True)

    with tile.TileContext(nc) as tc, ExitStack() as ctx:
        wpool = ctx.enter_context(tc.tile_pool(name="w", bufs=1))
        x8pool = ctx.enter_context(tc.tile_pool(name="x8", bufs=2))
        apool = ctx.enter_context(tc.tile_pool(name="act", bufs=3))
        vpool = ctx.enter_context(tc.tile_pool(name="vv", bufs=3))
        cpool = ctx.enter_context(tc.tile_pool(name="cum", bufs=6))
        opool = ctx.enter_context(tc.tile_pool(name="out", bufs=3))
        pp = ctx.enter_context(tc.tile_pool(name="pqk", bufs=3, space="PSUM"))
        pvp = ctx.enter_context(tc.tile_pool(name="pv", bufs=2, space="PSUM"))

        # --- resident weights + first-chunk fp16 x ---
        w16_t, w8_t, x16_t = [], [], []
        for i in range(3):
            t = wpool.tile([P, ND, EH], FP16, tag=f"w16_{i}")
            nc.sync.dma_start(
                out=t[:], in_=w16[i][:].rearrange("(j p) e -> p j e", p=P)
            )
            w16_t.append(t)
            t8 = wpool.tile([P, ND, EH], FP8, tag=f"w8_{i}")
            nc.sync.dma_start(
                out=t8[:], in_=w8[i][:].rearrange("(j p) e -> p j e", p=P)
            )
            w8_t.append(t8)
            tx = wpool.tile([P, ND, SC], FP16, tag=f"x16_{i}")
            nc.sync.dma_start(
                out=tx[:], in_=x16[i][:].rearrange("(j p) s -> p j s", p=P)
            )
            x16_t.append(tx)

        carry_k = [None] * N_EC
        carry_kv = [None] * N_EC

        def emit_unit(sc, ec, xts, fp8):
            e0 = ec * P
            sscale = (1.0 / WS) if fp8 else 1.0
            pqk = pp.tile([P, 2 * SC], FP32, tag="pqk")
            pv_ = pvp.tile([P, SC], FP32, tag="pv")
            for i in range(3):
                dst = pqk[:, i * SC : (i + 1) * SC] if i < 2 else pv_[:]
                if fp8:
                    for j in range(ND // 2):
                        nc.tensor.matmul(
                            dst,
                            lhsT=w8_t[i][:, 2 * j : 2 * j + 2, e0 : e0 + P],
                            rhs=xts[i][:, 2 * j : 2 * j + 2, :],
                            start=(j == 0),
                            stop=(j == ND // 2 - 1),
                            perf_mode=DR,
                            skip_group_check=True,
                        )
                else:
                    for j in range(ND):
                        nc.tensor.matmul(
                            dst,
                            lhsT=w16_t[i][:, j, e0 : e0 + P],
                            rhs=xts[i][:, j, :],
                            start=(j == 0),
                            stop=(j == ND - 1),
                            skip_group_check=True,
                        )
            # elu(x)+1 = exp(min(x,0)) + relu(x); the exp(min(x,0)) =
            # exp(-relu(-x)) chain runs entirely on ACT (which has headroom)
            # so the DVE pays only one 2x-rate add instead of a 1x-rate STT.
            rn = apool.tile([P, 2 * SC], FP16, tag="rn")
            nc.scalar.activation(rn[:], pqk[:], AF.Relu, scale=-sscale)
            ex = apool.tile([P, 2 * SC], FP16, tag="ex")
            nc.scalar.activation(ex[:], rn[:], AF.Exp, scale=-1.0)
            rp = apool.tile([P, 2 * SC], FP16, tag="rp")
            nc.scalar.activation(rp[:], pqk[:], AF.Relu, scale=sscale)
            v1 = vpool.tile([P, SC], FP16, tag="v1")
            nc.scalar.activation(v1[:], pv_[:], AF.Copy, scale=sscale)
            qk1 = apool.tile([P, 2 * SC], FP16, tag="qk1")
            nc.vector.tensor_add(qk1[:], ex[:], rp[:])
            q1 = qk1[:, 0:SC]
            k1 = qk1[:, SC : 2 * SC]
            kv = vpool.tile([P, SC], FP16, tag="kv")
            (nc.gpsimd if gps else nc.vector).tensor_mul(kv[:], k1, v1[:])
            # inclusive cumsums along s (free dim), chained across s-chunks
            ck = cpool.tile([P, SC], FP16, tag="ck")
            ckv = cpool.tile([P, SC], FP16, tag="ckv")
            if scan_mode == "scan":
                ik = 0.0 if sc == 0 else carry_k[ec][:, SC - 1 : SC]
                ikv = 0.0 if sc == 0 else carry_kv[ec][:, SC - 1 : SC]
                nc.vector.tensor_tensor_scan(
                    ck[:], k1, k1, ik, op0=AluOpType.add, op1=AluOpType.bypass
                )
                nc.vector.tensor_tensor_scan(
                    ckv[:], kv[:], kv[:], ikv, op0=AluOpType.add, op1=AluOpType.bypass
                )
            else:  # timing probe only
                nc.vector.tensor_copy(ck[:], k1)
                nc.vector.tensor_copy(ckv[:], kv[:])
            carry_k[ec], carry_kv[ec] = ck, ckv
            # out = q1 * (kv + ckv) / ck
            num = vpool.tile([P, SC], FP16, tag="num")
            (nc.gpsimd if gps else nc.vector).tensor_add(num[:], ckv[:], kv[:])
            t1 = vpool.tile([P, SC], FP16, tag="t1")
            nc.vector.tensor_mul(t1[:], q1, num[:])
            ot = opool.tile([P, SC], FP16, tag="ot")
            if div:
                nc.vector.tensor_tensor(ot[:], t1[:], ck[:], AluOpType.divide)
            else:
                den = vpool.tile([P, SC], FP16, tag="den")
                with nc.allow_low_precision(reason="1/k_prefix; 5e-4 rel ok"):
                    nc.vector.reciprocal(den[:], ck[:])
                nc.vector.tensor_mul(ot[:], t1[:], den[:])
            nc.sync.dma_start(
                out=outp[e0 : e0 + P, sc * SC : (sc + 1) * SC], in_=ot[:]
            )

        def main_body():
            for sc in range(N_SC):
                if sc == 0:
                    xts, fp8 = x16_t, False
                else:
                    xts = []
                    for i in range(3):
                        t = x8pool.tile([P, ND, SC], FP8, tag=f"x8{i}")
                        nc.sync.dma_start(
                            out=t[:],
                            in_=x8[i][:, sc * SC : (sc + 1) * SC].rearrange(
                                "(j p) s -> p j s", p=P
                            ),
                        )
                        xts.append(t)
                    fp8 = True
                for ec in range(N_EC):
                    emit_unit(sc, ec, xts, fp8)

        if repeat == 1:
            main_body()
        else:
            with tc.For_i(0, repeat, 1):
                main_body()

    nc.compile()
    return nc


def _e4m3(x):
    return np.clip(x, -240, 240).astype(ml_dtypes.float8_e4m3)


def _host_prep(v, k, q, Wq, Wk, Wv):
    """Build the 8 per-core input maps (x^T computed once per batch)."""
    xT8, xT16 = {}, {}
    for b in range(B):
        for n, x in (("q", q), ("k", k), ("v", v)):
            t = np.ascontiguousarray(x[b].T)  # [D, S] fp32
            xT8[(b, n)] = _e4m3(t)
            xT16[(b, n)] = t[:, :SC].astype(np.float16)
    in_maps = []
    for c in range(NCORES):
        b, h = c // 2, c % 2
        e0 = h * EH
        m = {}
        for n in ("q", "k", "v"):
            m[f"x{n}8"] = xT8[(b, n)]
            m[f"x{n}16"] = xT16[(b, n)]
        for n, W in (("q", Wq), ("k", Wk), ("v", Wv)):
            wt = np.ascontiguousarray(W.T[:, e0 : e0 + EH])
            m[f"w{n}16"] = wt.astype(np.float16)
            m[f"w{n}8"] = _e4m3(wt * WS)
        in_maps.append(m)
    return in_maps


_NC_CACHE = None


def _get_nc():
    global _NC_CACHE
    if _NC_CACHE is None:
        _NC_CACHE = build_nc()
    return _NC_CACHE


def run_spmd(v, k, q, Wq, Wk, Wv, **kwargs):
    """Run on 8 cores; returns (assembled output [B,S,E] fp32, raw results)."""
    nc = _get_nc()
    in_maps = _host_prep(v, k, q, Wq, Wk, Wv)
    res = run_bass_kernel_spmd(nc, in_maps, core_ids=list(range(NCORES)), **kwargs)
    full = np.empty((B, S, E), dtype=np.float32)
    for c in range(NCORES):
        b, h = c // 2, c % 2
        full[b, :, h * EH : (h + 1) * EH] = res.results[c]["out"].T.astype(np.float32)
    return full, res


def kernel(v, k, q, Wq, Wk, Wv):
    v, k, q, Wq, Wk, Wv = (
        np.asarray(a, dtype=np.float32) for a in (v, k, q, Wq, Wk, Wv)
    )
    full, _ = run_spmd(v, k, q, Wq, Wk, Wv)
    return full
